# revision 21
# baseline (speedup 1.0000x reference)
"""Multi-head self-attention (B=4, S=2048, E=1024, H=16, causal) on 8 NeuronCores.

Sharding (Megatron-style): data-parallel over B (4) x tensor-parallel over
heads (2 groups of 8). Core c handles batch c//2 with head-group c%2; the host
sums each pair of partial out-projections and adds bo.

Schedule (v2): the kernel is paced by ScalarE's exp (~182us busy); the PE must
never stall or the HAM clock-gate halves its rate. Query chunks run in
ascending order (qc 0..3, hp inner) so V tiles and QT/KT column chunks stream
just-in-time, and the dense fillers (QKV projections, out-projection) are
spread across phases sized to each phase's exp-pacing deficit:
  prologue: QK(qc0), V[0..3]   qc0: QK(qc1)          qc1: V[4..7], QK(qc2)
  qc2: V[8..11], QK(qc3), Wo(qc0)   qc3: V[12..15], Wo(qc1), Wo(qc2)
  tail: Wo(qc3)
Attention rounds software-pipeline AV one round behind scores/exp, the
softmax normalization is deferred off the PE critical path (pav is copied to
SBUF to free its PSUM bank, reciprocal_approx_fast on DVE, the rank-1
broadcast matmul lands one round into the next head-pair), causal mask
multiplies run on the idle GpSimd engine, and Q/K bias-adds moved to DVE so
ScalarE does exp only.
"""

import numpy as np
import ml_dtypes

B, S, E, H, D = 4, 2048, 1024, 16, 64
HPC = 8          # heads per core
DC = HPC * D     # 512 sharded feature cols per core
EC = E // 128    # 8 e-chunks
TT = S // 128    # 16 token tiles
QCH = S // 512   # 4 query chunks
NB = S // 128    # 16 key blocks

BF16 = ml_dtypes.bfloat16

_CACHE = {}

# debt model constants (ns): PE matmul ~0.42ns/col warm, group = 8 MMs
MM_NS_PER_COL = 0.42
MM_OVH = 25.0
GROUP_NS = 8 * (512 * MM_NS_PER_COL + MM_OVH)
ACT_OVH = 352.0 / 1.2   # per-activation fixed cost (ns)
ACT_PER = 1.0 / 1.2     # per free-element cost (ns)


def _build():
    import concourse.tile as tile
    from concourse import bacc, mybir

    F32 = mybir.dt.float32
    BF = mybir.dt.bfloat16
    AF = mybir.ActivationFunctionType
    ALU = mybir.AluOpType

    nc = bacc.Bacc("TRN2", target_bir_lowering=False, debug=False, num_devices=8)

    xT_d = nc.dram_tensor("xT", [128, EC, S], BF, kind="ExternalInput")
    wq_d = nc.dram_tensor("wq", [128, EC, DC], BF, kind="ExternalInput")
    wk_d = nc.dram_tensor("wk", [128, EC, DC], BF, kind="ExternalInput")
    wv_d = nc.dram_tensor("wv", [128, EC, DC], BF, kind="ExternalInput")
    wo_d = nc.dram_tensor("wo", [128, DC // 128, E], BF, kind="ExternalInput")
    bq_d = nc.dram_tensor("bq", [128, 4], F32, kind="ExternalInput")
    bk_d = nc.dram_tensor("bk", [128, 4], F32, kind="ExternalInput")
    bv_d = nc.dram_tensor("bv", [128, 4], F32, kind="ExternalInput")
    mask_d = nc.dram_tensor("mask", [128, 128], BF, kind="ExternalInput")
    out_d = nc.dram_tensor("out", [TT, 128, E], F32, kind="ExternalOutput")

    with tile.TileContext(nc) as tc:
        with tc.tile_pool(name="const", bufs=1) as cp, \
             tc.tile_pool(name="expp", bufs=1) as expp, \
             tc.tile_pool(name="work", bufs=2) as wp, \
             tc.tile_pool(name="ps_s", bufs=2, space="PSUM") as ps_s, \
             tc.tile_pool(name="ps_av", bufs=2, space="PSUM") as ps_av, \
             tc.tile_pool(name="ps_w", bufs=2, space="PSUM") as ps_w:

            # ---- persistent SBUF tensors (one tile per tensor: one DMA each) ----
            xT = cp.tile([128, EC, S], BF, tag="xT", name="xT")
            wq = cp.tile([128, EC, DC], BF, tag="wq", name="wq")
            wk = cp.tile([128, EC, DC], BF, tag="wk", name="wk")
            wv = cp.tile([128, EC, DC], BF, tag="wv", name="wv")
            wo = cp.tile([128, DC // 128, E], BF, tag="wo", name="wo")
            bq = cp.tile([128, 4], F32, tag="bq", name="bq")
            bk = cp.tile([128, 4], F32, tag="bk", name="bk")
            bv = cp.tile([128, 4], F32, tag="bv", name="bv")
            mask = cp.tile([128, 2, 128], BF, tag="mask", name="mask")
            ones = cp.tile([65, 64], BF, tag="ones", name="ones")
            warm = cp.tile([1, 16], F32, tag="warm", name="warm")

            # small inputs + x (qc0 chunk first) on the sync queue; weights
            # spread over the other two queues. The t=0 columns of wk land
            # first so the first Q/K projection starts ASAP.
            nc.sync.dma_start(bq[:], bq_d.ap())
            nc.sync.dma_start(bk[:], bk_d.ap())
            nc.sync.dma_start(bv[:], bv_d.ap())
            nc.sync.dma_start(mask[:, 0, :], mask_d.ap())
            nc.sync.dma_start(mask[:, 1, :], mask_d.ap())
            nc.sync.dma_start(xT[:, 0:4, 0:512], xT_d.ap()[:, 0:4, 0:512])
            nc.sync.dma_start(xT[:, 4:8, 0:512], xT_d.ap()[:, 4:8, 0:512])
            nc.sync.dma_start(xT[:, :, 512:1024], xT_d.ap()[:, :, 512:1024])
            nc.sync.dma_start(xT[:, :, 1024:S], xT_d.ap()[:, :, 1024:S])
            nc.gpsimd.dma_start(wq[:, :, 0:128], wq_d.ap()[:, :, 0:128])
            nc.gpsimd.dma_start(wq[:, :, 128:DC], wq_d.ap()[:, :, 128:DC])
            nc.gpsimd.dma_start(wo[:], wo_d.ap())
            nc.scalar.dma_start(wk[:, :, 0:128], wk_d.ap()[:, :, 0:128])
            nc.scalar.dma_start(wv[:], wv_d.ap())
            nc.scalar.dma_start(wk[:, :, 128:DC], wk_d.ap()[:, :, 128:DC])

            nc.any.memset(ones[:], 1.0)
            # preload the exp LUT during the DMA prologue
            nc.any.memset(warm[:, 0:8], 0.0)
            nc.scalar.activation(warm[:, 8:16], warm[:, 0:8], AF.Exp, scale=1.0)

            QT = [cp.tile([128, S], BF, tag=f"QT{t}", name=f"QT{t}") for t in range(4)]
            KT = [cp.tile([128, S], BF, tag=f"KT{t}", name=f"KT{t}") for t in range(4)]
            V = [cp.tile([128, HPC, 66], BF, tag=f"V{s}", name=f"V{s}") for s in range(TT)]
            AOT = [cp.tile([128, S], BF, tag=f"AOT{t}", name=f"AOT{t}") for t in range(4)]

            # ---- dense work groups (PE fillers) ----
            def proj_group(w_sb, b_sb, dst, t, qc):
                def emit():
                    ps = ps_w.tile([128, 512], F32, tag="psw", name="psw")
                    for k in range(EC):
                        nc.tensor.matmul(
                            ps[:],
                            w_sb[:, k, t * 128:(t + 1) * 128],
                            xT[:, k, qc * 512:(qc + 1) * 512],
                            start=(k == 0), stop=(k == EC - 1))
                    nc.vector.tensor_scalar(
                        dst[t][:, qc * 512:(qc + 1) * 512], ps[:],
                        b_sb[:, t:t + 1], None, ALU.add)
                return emit

            def v_group(s):
                def emit():
                    ps = ps_w.tile([128, 512], F32, tag="psw", name="psw")
                    for k in range(EC):
                        nc.tensor.matmul(
                            ps[:],
                            xT[:, k, s * 128:(s + 1) * 128],
                            wv[:, k, :],
                            start=(k == 0), stop=(k == EC - 1))
                    nc.vector.tensor_copy(
                        out=V[s][:, :, 0:64],
                        in_=ps[:].rearrange("p (h d) -> p h d", d=64))
                    nc.any.memset(V[s][:, :, 64:65], 1.0)
                return emit

            def d_group(s):
                def emit():
                    osb = wp.tile([128, E], F32, tag="osb", name="osb")
                    for n in range(2):
                        ps = ps_w.tile([128, 512], F32, tag="psw", name="psw")
                        for k in range(DC // 128):
                            nc.tensor.matmul(
                                ps[:],
                                AOT[k][:, s * 128:(s + 1) * 128],
                                wo[:, k, n * 512:(n + 1) * 512],
                                start=(k == 0), stop=(k == DC // 128 - 1))
                        nc.vector.tensor_copy(out=osb[:, n * 512:(n + 1) * 512],
                                              in_=ps[:])
                    nc.sync.dma_start(out_d.ap()[s], osb[:])
                return emit

            # ---- prologue PE work: just enough for (hp0, qc0) round 0 ----
            proj_group(wq, bq, QT, 0, 0)()
            proj_group(wk, bk, KT, 0, 0)()
            v_group(0)()
            v_group(1)()

            # ---- filler queue (kind, idx, emit) ----
            # kinds: "v" s-tile (forced before the round using key block s),
            # "qk0" pair-t qc0 projections (forced before hp=t of qc0),
            # "qk" next-chunk projections (forced at chunk start), "d" out-proj
            filler = []
            state = {"emitted": 0, "target0": 0, "rounds": 1, "ridx": 0}

            def qk_items(qc):
                its = []
                for t in range(4):
                    its.append(("qk", qc, proj_group(wq, bq, QT, t, qc)))
                    its.append(("qk", qc, proj_group(wk, bk, KT, t, qc)))
                return its

            phase_fillers = {
                0: [("v", 2, v_group(2)), ("v", 3, v_group(3))]
                   + [("qk0", t, proj_group(w, b, dst, t, 0))
                      for t in (1, 2, 3)
                      for w, b, dst in ((wq, bq, QT), (wk, bk, KT))]
                   + qk_items(1),
                1: [("v", s, v_group(s)) for s in range(4, 8)] + qk_items(2),
                2: [("v", s, v_group(s)) for s in range(8, 12)] + qk_items(3)
                   + [("d", s, d_group(s)) for s in range(0, 4)],
                3: [("v", s, v_group(s)) for s in range(12, 16)]
                   + [("d", s, d_group(s)) for s in range(4, 12)],
            }

            def start_phase(qc):
                filler.extend(phase_fillers[qc])
                state["emitted"] = 0
                state["target0"] = len(filler)
                state["rounds"] = 4 * (4 * qc + 4)
                state["ridx"] = 0

            def force(pred):
                keep = []
                for it in filler:
                    if pred(it):
                        it[2]()
                        state["emitted"] += 1
                    else:
                        keep.append(it)
                filler[:] = keep

            def emit_filler_quota():
                # even pacing, aiming to drain six key-block rounds early
                state["ridx"] += 1
                eff = max(1, state["rounds"] - 6)
                target = state["target0"] * min(state["ridx"], eff) // eff
                while state["emitted"] < target and filler:
                    filler.pop(0)[2]()
                    state["emitted"] += 1

            # ---- attention ----
            pending_norm = []

            for qc in range(QCH):
                start_phase(qc)
                force(lambda it: it[0] == "qk" and it[1] <= qc)
                nkb = 4 * qc + 4
                for hp in range(4):
                    if qc == 0:
                        force(lambda it: it[0] == "qk0" and it[1] <= hp)
                    hA, hB = 2 * hp, 2 * hp + 1
                    pav = {h: ps_av.tile([128, 512], F32, tag="pav", name="pav")
                           for h in (hA, hB)}
                    # both heads share one expT tile so a single exp
                    # instruction covers the pair (dim1 = head)
                    expT = expp.tile([128, 2, NB, 512], BF, tag="expT",
                                     name="expT")

                    def emit_av(kb_off):
                        kb, off = kb_off
                        for hi, h in ((0, hA), (1, hB)):
                            nc.tensor.matmul(
                                pav[h][0:65, off:512],
                                V[kb][:, h, 0:65],
                                expT[:, hi, kb, off:512],
                                start=(kb == 0), stop=(kb == nkb - 1))

                    # key blocks in pairs: the 4 scores MMs, the 2 exps, and
                    # the 4 lagged AV MMs are each emitted as one block to
                    # minimize PE row-configuration switches
                    pending_av = []
                    for p in range(nkb // 2):
                        kbs = (2 * p, 2 * p + 1)
                        force(lambda it: it[0] == "v" and it[1] <= kbs[1])
                        pss = {}
                        offs = {}
                        for kb in kbs:
                            pss[kb] = ps_s.tile([128, 2, 512], F32, tag="pss",
                                                name="pss")
                            dj = kb - 4 * qc
                            offs[kb] = 128 * dj if dj > 0 else 0
                        for kb in kbs:
                            off = offs[kb]
                            for hi, (h, r) in enumerate(((hA, 0), (hB, 64))):
                                nc.tensor.matmul(
                                    pss[kb][:, hi, off:512],
                                    KT[hp][r:r + 64, kb * 128:(kb + 1) * 128],
                                    QT[hp][r:r + 64,
                                           qc * 512 + off:(qc + 1) * 512],
                                    start=True, stop=True)
                        for kb in kbs:
                            off = offs[kb]
                            nc.scalar.activation(
                                expT[:, 0:2, kb, off:512],
                                pss[kb][:, 0:2, off:512],
                                AF.Exp, scale=0.125)
                            if kb - 4 * qc >= 0:
                                nc.gpsimd.tensor_tensor(
                                    expT[:, 0:2, kb, off:off + 128],
                                    expT[:, 0:2, kb, off:off + 128],
                                    mask[:], ALU.mult)
                        # deferred norm of the previous head-pair lands here,
                        # before this pair's first AV (its rcp is ready by now
                        # and AV must not precede it: pav pool slot reuse)
                        if p == 1 and pending_norm:
                            for fn in pending_norm:
                                fn()
                            pending_norm.clear()
                        # AV runs one pair behind scores/exp so the PE never
                        # waits on ScalarE
                        for kb_off in pending_av:
                            emit_av(kb_off)
                        pending_av = [(kb, offs[kb]) for kb in kbs]
                        emit_filler_quota()
                        emit_filler_quota()
                    for kb_off in pending_av:
                        emit_av(kb_off)
                    pending_av = []

                    # free pav PSUM quickly, start the reciprocal on DVE
                    # (reciprocal_approx_fast only works at base partition 0,
                    # so the denominator row is copy-shifted 64->0 first); the
                    # PE-side broadcast + the normalize multiplies are deferred
                    pavcp = {}
                    rcpbf = {}
                    for h in (hA, hB):
                        pavcp[h] = wp.tile([65, 512], F32, tag="pavcp",
                                           name="pavcp")
                        nc.vector.tensor_copy(out=pavcp[h][:], in_=pav[h][0:65, :])
                        den0 = wp.tile([1, 512], F32, tag="den0", name="den0")
                        nc.vector.tensor_copy(out=den0[0:1, :],
                                              in_=pav[h][64:65, :])
                        rcp32 = wp.tile([1, 512], F32, tag="rcp32", name="rcp32")
                        nc.vector.reciprocal_approx_fast(
                            out=rcp32[0:1, :], in_=den0[0:1, :])
                        rcpbf[h] = wp.tile([1, 512], BF, tag="rcpbf",
                                           name="rcpbf")
                        nc.vector.tensor_copy(out=rcpbf[h][0:1, :],
                                              in_=rcp32[0:1, :])

                    def norm_rest(hp=hp, qc=qc, pavcp=pavcp, rcpbf=rcpbf,
                                  hA=hA, hB=hB):
                        for h, r in ((hA, 0), (hB, 64)):
                            psb = ps_w.tile([128, 512], F32, tag="psw",
                                            name="psw")
                            nc.tensor.matmul(psb[0:64, :], ones[0:1, :],
                                             rcpbf[h][0:1, :],
                                             start=True, stop=True)
                            dst = AOT[hp][r:r + 64, qc * 512:(qc + 1) * 512]
                            nc.vector.tensor_tensor(dst, pavcp[h][0:64, :],
                                                    psb[0:64, :], ALU.mult)
                            nc.vector.tensor_scalar(dst, dst,
                                                    bv[r:r + 64, hp:hp + 1],
                                                    None, ALU.add)
                    pending_norm.append(norm_rest)

            # ---- tail ----
            for fn in pending_norm:
                fn()
            pending_norm.clear()
            for it in filler:
                it[2]()
            filler.clear()
            for s in range(12, 16):
                d_group(s)()

    nc.compile()
    return nc


def _get_nc():
    if "nc" not in _CACHE:
        _CACHE["nc"] = _build()
    return _CACHE["nc"]


def _shard_inputs(x, Wq, bq, Wk, bk, Wv, bv, Wo):
    """Build the 8 per-core input maps (host-side shard/cast/transpose)."""
    x = np.asarray(x, np.float32)
    mask = np.triu(np.ones((128, 128), np.float32)).astype(BF16)  # [k, q] q>=k
    in_maps = []
    for c in range(8):
        b, hg = divmod(c, 2)
        dc = slice(hg * DC, (hg + 1) * DC)
        xT = np.ascontiguousarray(
            x[b].T.astype(BF16).reshape(EC, 128, S).transpose(1, 0, 2))
        wq_c = np.ascontiguousarray(
            Wq[:, dc].astype(BF16).reshape(EC, 128, DC).transpose(1, 0, 2))
        wk_c = np.ascontiguousarray(
            Wk[:, dc].astype(BF16).reshape(EC, 128, DC).transpose(1, 0, 2))
        wv_c = np.ascontiguousarray(
            Wv[:, dc].astype(BF16).reshape(EC, 128, DC).transpose(1, 0, 2))
        wo_c = np.ascontiguousarray(
            Wo[dc, :].astype(BF16).reshape(DC // 128, 128, E).transpose(1, 0, 2))
        bq_c = np.ascontiguousarray(np.asarray(bq[dc], np.float32).reshape(4, 128).T)
        bk_c = np.ascontiguousarray(np.asarray(bk[dc], np.float32).reshape(4, 128).T)
        bv_c = np.ascontiguousarray(np.asarray(bv[dc], np.float32).reshape(4, 128).T)
        in_maps.append({
            "xT": xT, "wq": wq_c, "wk": wk_c, "wv": wv_c, "wo": wo_c,
            "bq": bq_c, "bk": bk_c, "bv": bv_c, "mask": mask,
        })
    return in_maps


def kernel(x, Wq, bq, Wk, bk, Wv, bv, Wo, bo):
    from concourse.bass_utils import run_bass_kernel_spmd

    nc = _get_nc()
    in_maps = _shard_inputs(x, Wq, bq, Wk, bk, Wv, bv, Wo)
    res = run_bass_kernel_spmd(nc, in_maps, core_ids=list(range(8)))
    bo = np.asarray(bo, np.float32)
    out = np.empty((B, S, E), np.float32)
    for b in range(B):
        p0 = res.results[2 * b]["out"].reshape(S, E)
        p1 = res.results[2 * b + 1]["out"].reshape(S, E)
        out[b] = p0 + p1 + bo
    return out
